# revision 7
# baseline (speedup 1.0000x reference)
"""Gumbel vector quantizer on 8 trn2 cores.

Shapes (hardcoded):
  hidden_states (16, 2048, 512) f32
  W             (640, 512) f32, b (640,) f32
  code_book     (1, 2, 320, 384) f32
  gumbel_noise  (65536, 320) f32   # row = (b*2048 + l)*2 + g

Outputs: code_vectors (16, 2048, 768), perplexity (2048, 2, 320)

Forward-pass identities used:
  probs == y_hard (straight-through is identity in fwd)  -> pure argmax + gather
  argmax(softmax((z+g)/tau)) == argmax(z + g)            -> no exp on noisy path

Sharding: data-parallel over batch, 2 batches (4096 tokens) per core.
Perplexity partial sums returned per core, summed on host.
"""

from contextlib import ExitStack

import numpy as np

import concourse.bacc as bacc
import concourse.bass as bass
import concourse.mybir as mybir
import concourse.tile as tile
from concourse import bass_utils
from concourse.masks import make_identity

B, L, D = 16, 2048, 512
G, V, DG = 2, 320, 384
GV = G * V            # 640
CVD = G * DG          # 768
NCORES = 8
BPC = B // NCORES     # batches per core = 2
TOK = BPC * L         # tokens per core = 4096
NT = TOK // 128       # token tiles per core = 32
HALF = NT // 2        # tiles per batch = 16
KC = D // 128         # contraction chunks = 4

FP32 = mybir.dt.float32
U32 = mybir.dt.uint32

_cache = {}


def _build():
    nc = bacc.Bacc("TRN2", target_bir_lowering=False, debug=False,
                   enable_asserts=False, num_devices=NCORES)

    x_d = nc.dram_tensor("x", [TOK, D], FP32, kind="ExternalInput").ap()
    gn_d = nc.dram_tensor("gn", [TOK, GV], FP32, kind="ExternalInput").ap()
    wt_d = nc.dram_tensor("wt", [D, GV], FP32, kind="ExternalInput").ap()
    cb_d = [nc.dram_tensor(f"cb{g}", [V, DG], FP32, kind="ExternalInput").ap()
            for g in range(G)]
    cv_d = nc.dram_tensor("cv", [TOK, CVD], FP32, kind="ExternalOutput").ap()
    pp_d = nc.dram_tensor("pp", [L, GV], FP32, kind="ExternalOutput").ap()

    with tile.TileContext(nc) as tc:
        ctx = ExitStack()
        with ctx:
            const = ctx.enter_context(tc.tile_pool(name="const", bufs=1))
            wpool = ctx.enter_context(tc.tile_pool(name="wpool", bufs=1))
            xp = ctx.enter_context(tc.tile_pool(name="xp", bufs=3))
            xtp = ctx.enter_context(tc.tile_pool(name="xtp", bufs=3))
            gp = ctx.enter_context(tc.tile_pool(name="gp", bufs=3))
            zp = ctx.enter_context(tc.tile_pool(name="zp", bufs=3))
            ep = ctx.enter_context(tc.tile_pool(name="ep", bufs=3))
            cvp = ctx.enter_context(tc.tile_pool(name="cvp", bufs=3))
            ppp = ctx.enter_context(tc.tile_pool(name="ppp", bufs=2))
            smp = ctx.enter_context(tc.tile_pool(name="smp", bufs=6))
            ps_t = ctx.enter_context(tc.tile_pool(name="ps_t", bufs=2, space="PSUM"))
            ps_z = ctx.enter_context(tc.tile_pool(name="ps_z", bufs=4, space="PSUM"))

            ident = const.tile([128, 128], FP32)
            make_identity(nc, ident[:])

            # W^T resident in SBUF: [128, KC, GV] -> chunk k is wt[:, k, :]
            wt = wpool.tile([128, KC, GV], FP32)
            nc.sync.dma_start(wt[:], wt_d.rearrange("(k p) o -> p k o", p=128))

            def half_softmax(i, g, z, e, r):
                """exp(z_g - max) into e[:, g, :], 1/(16*sum) into r (col g)."""
                zg = z[:, g, :]
                negm = smp.tile([128, 1], FP32, tag="negm")
                nc.vector.tensor_reduce(negm[:], zg, axis=mybir.AxisListType.X,
                                        op=mybir.AluOpType.max, negate=True)
                s = smp.tile([128, 1], FP32, tag="s")
                nc.scalar.activation(e[:, g, :], zg,
                                     mybir.ActivationFunctionType.Exp,
                                     bias=negm[:], accum_out=s[:])
                s16 = smp.tile([128, 1], FP32, tag="s16")
                nc.vector.tensor_scalar_mul(s16[:], s[:], float(B))
                nc.vector.reciprocal(r[:, g : g + 1], s16[:])

            def do_tile(i):
                """Full pipeline for token tile i; returns (e, r) for perp."""
                # load x tile, transpose to [d, tok] chunks
                x = xp.tile([128, D], FP32, tag="x")
                nc.sync.dma_start(x[:], x_d[i * 128 : (i + 1) * 128, :])
                xt = xtp.tile([128, KC, 128], FP32, tag="xt")
                pt = ps_t.tile([128, KC, 128], FP32, tag="pt", space="PSUM")
                for k in range(KC):
                    nc.tensor.transpose(pt[:, k, :], x[:, k * 128 : (k + 1) * 128],
                                        ident[:])
                nc.vector.tensor_copy(xt[:], pt[:])

                # logits: out[tok, gv] += xt_k.T @ wt_k ; one PSUM bank per group
                z = zp.tile([128, G, V], FP32, tag="z")
                for g in range(G):
                    pz = ps_z.tile([128, V], FP32, tag="pz", space="PSUM")
                    for k in range(KC):
                        nc.tensor.matmul(pz[:], xt[:, k, :],
                                         wt[:, k, g * V : (g + 1) * V],
                                         start=(k == 0), stop=(k == KC - 1))
                    nc.vector.tensor_copy(z[:, g, :], pz[:])

                # noisy argmax path (gumbel already has bias b folded in, host)
                gn = gp.tile([128, G, V], FP32, tag="gn")
                nc.sync.dma_start(gn[:], gn_d[i * 128 : (i + 1) * 128, :]
                                  .rearrange("p (g v) -> p g v", g=G))
                nc.vector.tensor_add(gn[:], gn[:], z[:])

                cv = cvp.tile([128, G, DG], FP32, tag="cv")
                for g in range(G):
                    mx = smp.tile([128, 8], FP32, tag="mx")
                    idx = smp.tile([128, 8], U32, tag="idx")
                    nc.vector.max(mx[:], gn[:, g, :])
                    nc.vector.max_index(idx[:], mx[:], gn[:, g, :])
                    nc.gpsimd.indirect_dma_start(
                        out=cv[:, g, :], out_offset=None,
                        in_=cb_d[g][:],
                        in_offset=bass.IndirectOffsetOnAxis(ap=idx[:, 0:1], axis=0),
                    )
                nc.sync.dma_start(cv_d[i * 128 : (i + 1) * 128, :],
                                  cv[:].rearrange("p g v -> p (g v)"))

                # softmax for perplexity
                e = ep.tile([128, G, V], FP32, tag="e")
                r = smp.tile([128, G], FP32, tag="r")
                for g in range(G):
                    half_softmax(i, g, z, e, r)
                return e, r

            for i in range(HALF):
                e0, r0 = do_tile(i)
                e1, r1 = do_tile(i + HALF)
                pp = ppp.tile([128, G, V], FP32, tag="pp")
                for g in range(G):
                    nc.vector.tensor_scalar_mul(pp[:, g, :], e1[:, g, :],
                                                r1[:, g : g + 1])
                    nc.vector.scalar_tensor_tensor(
                        out=pp[:, g, :], in0=e0[:, g, :],
                        scalar=r0[:, g : g + 1], in1=pp[:, g, :],
                        op0=mybir.AluOpType.mult, op1=mybir.AluOpType.add)
                nc.sync.dma_start(pp_d[i * 128 : (i + 1) * 128, :],
                                  pp[:].rearrange("p g v -> p (g v)"))

    nc.compile()
    return nc


def _numpy_fallback(hs, W, b, cb, gn):
    logits = hs.reshape(B * L, D) @ W.T + b
    flat = logits.reshape(B * L * G, V)
    idx = np.argmax(flat + gn, axis=-1)
    cbf = cb.reshape(G, V, DG)
    cv = np.stack([cbf[g][idx.reshape(-1, G)[:, g]] for g in range(G)], axis=1)
    z = logits.reshape(B, L, G, V)
    ez = np.exp(z - z.max(-1, keepdims=True))
    soft = ez / ez.sum(-1, keepdims=True)
    return (cv.reshape(B, L, CVD).astype(np.float32),
            (soft.sum(0) / B).astype(np.float32))


def kernel(hidden_states, W, b, code_book, gumbel_noise):
    hs = np.ascontiguousarray(hidden_states, dtype=np.float32)
    cb = np.ascontiguousarray(code_book, dtype=np.float32)
    if np.any(np.asarray(b) != 0):
        # device path folds b only into the argmax branch; exact for b==0
        # (always true for this model's init). Anything else: host math.
        return _numpy_fallback(hs, np.asarray(W), np.asarray(b), cb,
                               np.asarray(gumbel_noise))

    if "nc" not in _cache:
        _cache["nc"] = _build()
    nc = _cache["nc"]

    wt = np.ascontiguousarray(np.asarray(W).T, dtype=np.float32)
    gnb = np.ascontiguousarray(gumbel_noise, dtype=np.float32).reshape(B * L, GV)

    in_maps = []
    for c in range(NCORES):
        in_maps.append({
            "x": hs[c * BPC : (c + 1) * BPC].reshape(TOK, D),
            "gn": gnb[c * TOK : (c + 1) * TOK],
            "wt": wt,
            "cb0": cb[0, 0],
            "cb1": cb[0, 1],
        })
    _cache["in_maps"] = in_maps

    res = bass_utils.run_bass_kernel_spmd(nc, in_maps, core_ids=list(range(NCORES)))

    cv = np.empty((B, L, CVD), dtype=np.float32)
    pp = np.zeros((L, G, V), dtype=np.float32)
    for c in range(NCORES):
        cv[c * BPC : (c + 1) * BPC] = res.results[c]["cv"].reshape(BPC, L, CVD)
        pp += res.results[c]["pp"].reshape(L, G, V)
    return cv, pp


# revision 10
# speedup vs baseline: 1.6686x; 1.6686x over previous
"""Gumbel vector quantizer on 8 trn2 cores.

Shapes (hardcoded):
  hidden_states (16, 2048, 512) f32
  W             (640, 512) f32, b (640,) f32
  code_book     (1, 2, 320, 384) f32
  gumbel_noise  (65536, 320) f32   # row = (b*2048 + l)*2 + g

Outputs: code_vectors (16, 2048, 768), perplexity (2048, 2, 320)

Forward-pass identities used:
  probs == y_hard (straight-through is identity in fwd)  -> pure argmax + gather
  argmax(softmax((z+g)/tau)) == argmax(z + g)            -> no exp on noisy path

Sharding: data-parallel over batch, 2 batches (4096 tokens) per core.
Perplexity partial sums returned per core, summed on host.
"""

from contextlib import ExitStack

import numpy as np

import concourse.bacc as bacc
import concourse.bass as bass
import concourse.mybir as mybir
import concourse.tile as tile
from concourse import bass_utils
from concourse.masks import make_identity

B, L, D = 16, 2048, 512
G, V, DG = 2, 320, 384
GV = G * V            # 640
CVD = G * DG          # 768
NCORES = 8
BPC = B // NCORES     # batches per core = 2
TOK = BPC * L         # tokens per core = 4096
NT = TOK // 128       # token tiles per core = 32
HALF = NT // 2        # tiles per batch = 16
KC = D // 128         # contraction chunks = 4

FP32 = mybir.dt.float32
U32 = mybir.dt.uint32

_cache = {}


def _build():
    nc = bacc.Bacc("TRN2", target_bir_lowering=False, debug=False,
                   enable_asserts=False, num_devices=NCORES)

    x_d = nc.dram_tensor("x", [TOK, D], FP32, kind="ExternalInput").ap()
    gn_d = nc.dram_tensor("gn", [TOK, GV], FP32, kind="ExternalInput").ap()
    wt_d = nc.dram_tensor("wt", [D, GV], FP32, kind="ExternalInput").ap()
    cb_d = [nc.dram_tensor(f"cb{g}", [V, DG], FP32, kind="ExternalInput").ap()
            for g in range(G)]
    cv_d = nc.dram_tensor("cv", [TOK, CVD], FP32, kind="ExternalOutput").ap()
    pp_d = nc.dram_tensor("pp", [L, GV], FP32, kind="ExternalOutput").ap()

    with tile.TileContext(nc) as tc:
        ctx = ExitStack()
        with ctx:
            const = ctx.enter_context(tc.tile_pool(name="const", bufs=1))
            wpool = ctx.enter_context(tc.tile_pool(name="wpool", bufs=1))
            xp = ctx.enter_context(tc.tile_pool(name="xp", bufs=3))
            xtp = ctx.enter_context(tc.tile_pool(name="xtp", bufs=3))
            gp = ctx.enter_context(tc.tile_pool(name="gp", bufs=3))
            ep = ctx.enter_context(tc.tile_pool(name="ep", bufs=4))
            cvp = ctx.enter_context(tc.tile_pool(name="cvp", bufs=3))
            ppp = ctx.enter_context(tc.tile_pool(name="ppp", bufs=2))
            smp = ctx.enter_context(tc.tile_pool(name="smp", bufs=6))
            ps_t = ctx.enter_context(tc.tile_pool(name="ps_t", bufs=2, space="PSUM"))
            ps_z = ctx.enter_context(tc.tile_pool(name="ps_z", bufs=3, space="PSUM"))

            ident = const.tile([128, 128], FP32)
            make_identity(nc, ident[:])

            # W^T resident in SBUF: [128, KC, GV] -> chunk k is wt[:, k, :]
            wt = wpool.tile([128, KC, GV], FP32)
            nc.sync.dma_start(wt[:], wt_d.rearrange("(k p) o -> p k o", p=128))

            def stage_a(i):
                """DMA x tile i, transpose to [d, tok] layout in SBUF."""
                x = xp.tile([128, D], FP32, tag="x")
                nc.sync.dma_start(x[:], x_d[i * 128 : (i + 1) * 128, :])
                pt = ps_t.tile([128, KC, 128], FP32, tag="pt", space="PSUM")
                for k in range(KC):
                    nc.tensor.transpose(pt[:, k, :], x[:, k * 128 : (k + 1) * 128],
                                        ident[:])
                xt = xtp.tile([128, KC, 128], FP32, tag="xt")
                nc.scalar.copy(xt[:], pt[:])
                return xt

            def stage_b(i, xt):
                """Matmuls + argmax/gather + softmax for token tile i."""
                pz = [ps_z.tile([128, V], FP32, tag=f"pz{g}", name=f"pz{g}",
                                space="PSUM") for g in range(G)]
                for g in range(G):
                    for k in range(KC):
                        nc.tensor.matmul(pz[g][:], xt[:, k, :],
                                         wt[:, k, g * V : (g + 1) * V],
                                         start=(k == 0), stop=(k == KC - 1))

                # noisy argmax path (bias b already folded into gn on host)
                gn = gp.tile([128, G, V], FP32, tag="gn")
                nc.sync.dma_start(gn[:], gn_d[i * 128 : (i + 1) * 128, :]
                                  .rearrange("p (g v) -> p g v", g=G))
                cv = cvp.tile([128, G, DG], FP32, tag="cv")
                for g in range(G):
                    nc.vector.tensor_add(gn[:, g, :], gn[:, g, :], pz[g][:])
                    mx = smp.tile([128, 8], FP32, tag="mx")
                    idx = smp.tile([128, 8], U32, tag="idx")
                    nc.vector.max(mx[:], gn[:, g, :])
                    nc.vector.max_index(idx[:], mx[:], gn[:, g, :])
                    nc.gpsimd.indirect_dma_start(
                        out=cv[:, g, :], out_offset=None,
                        in_=cb_d[g][:],
                        in_offset=bass.IndirectOffsetOnAxis(ap=idx[:, 0:1], axis=0),
                    )
                nc.sync.dma_start(cv_d[i * 128 : (i + 1) * 128, :],
                                  cv[:].rearrange("p g v -> p (g v)"))

                # softmax for perplexity; |z| < ~4 so exp(z) is safe unshifted
                e = ep.tile([128, G, V], FP32, tag="e")
                s = smp.tile([128, G], FP32, tag="s")
                for g in range(G):
                    nc.scalar.activation(e[:, g, :], pz[g][:],
                                         mybir.ActivationFunctionType.Exp,
                                         accum_out=s[:, g : g + 1])
                r = smp.tile([128, G], FP32, tag="r")
                nc.vector.tensor_scalar_mul(r[:], s[:], float(B))
                nc.vector.reciprocal(r[:], r[:])
                return e, r

            order = []
            for i in range(HALF):
                order += [i, i + HALF]
            xts = {order[0]: stage_a(order[0])}
            results = {}
            for j, i in enumerate(order):
                if j + 1 < len(order):
                    xts[order[j + 1]] = stage_a(order[j + 1])
                results[i] = stage_b(i, xts.pop(i))
                if i >= HALF:
                    e0, r0 = results.pop(i - HALF)
                    e1, r1 = results.pop(i)
                    pp = ppp.tile([128, G, V], FP32, tag="pp")
                    for g in range(G):
                        nc.vector.tensor_scalar_mul(pp[:, g, :], e1[:, g, :],
                                                    r1[:, g : g + 1])
                        nc.vector.scalar_tensor_tensor(
                            out=pp[:, g, :], in0=e0[:, g, :],
                            scalar=r0[:, g : g + 1], in1=pp[:, g, :],
                            op0=mybir.AluOpType.mult, op1=mybir.AluOpType.add)
                    li = i - HALF
                    nc.sync.dma_start(pp_d[li * 128 : (li + 1) * 128, :],
                                      pp[:].rearrange("p g v -> p (g v)"))

    nc.compile()
    return nc


def _numpy_fallback(hs, W, b, cb, gn):
    logits = hs.reshape(B * L, D) @ W.T + b
    flat = logits.reshape(B * L * G, V)
    idx = np.argmax(flat + gn, axis=-1)
    cbf = cb.reshape(G, V, DG)
    cv = np.stack([cbf[g][idx.reshape(-1, G)[:, g]] for g in range(G)], axis=1)
    z = logits.reshape(B, L, G, V)
    ez = np.exp(z - z.max(-1, keepdims=True))
    soft = ez / ez.sum(-1, keepdims=True)
    return (cv.reshape(B, L, CVD).astype(np.float32),
            (soft.sum(0) / B).astype(np.float32))


def kernel(hidden_states, W, b, code_book, gumbel_noise):
    hs = np.ascontiguousarray(hidden_states, dtype=np.float32)
    cb = np.ascontiguousarray(code_book, dtype=np.float32)
    if np.any(np.asarray(b) != 0):
        # device path folds b only into the argmax branch; exact for b==0
        # (always true for this model's init). Anything else: host math.
        return _numpy_fallback(hs, np.asarray(W), np.asarray(b), cb,
                               np.asarray(gumbel_noise))

    if "nc" not in _cache:
        _cache["nc"] = _build()
    nc = _cache["nc"]

    wt = np.ascontiguousarray(np.asarray(W).T, dtype=np.float32)
    gnb = np.ascontiguousarray(gumbel_noise, dtype=np.float32).reshape(B * L, GV)

    in_maps = []
    for c in range(NCORES):
        in_maps.append({
            "x": hs[c * BPC : (c + 1) * BPC].reshape(TOK, D),
            "gn": gnb[c * TOK : (c + 1) * TOK],
            "wt": wt,
            "cb0": cb[0, 0],
            "cb1": cb[0, 1],
        })
    _cache["in_maps"] = in_maps

    res = bass_utils.run_bass_kernel_spmd(nc, in_maps, core_ids=list(range(NCORES)))

    cv = np.empty((B, L, CVD), dtype=np.float32)
    pp = np.zeros((L, G, V), dtype=np.float32)
    for c in range(NCORES):
        cv[c * BPC : (c + 1) * BPC] = res.results[c]["cv"].reshape(BPC, L, CVD)
        pp += res.results[c]["pp"].reshape(L, G, V)
    return cv, pp
